# revision 35
# baseline (speedup 1.0000x reference)
"""Trainium2 Bass kernel for nn_MultiHeadAttention_6055903887702.

Sharding: one attention head per NeuronCore (H == n_cores == 8). Each core
computes, for its head h:
    A_h  = Wq_h Wk_h^T  (host-precomputed, so Q/K projections collapse)
    GT_h = A_h^T X^T, V_h = X Wv_h                              (f32r matmuls)
    ST_h = X G^T      (scores, transposed layout [t, s])        (f32r matmuls)
    P_h  = exp(ST_h / sqrt(E))   (unnormalized, no max-sub — logits ~N(0,1))
    colsum[s] = sum_t P_h[t, s]  (DVE accumulate + one f32r ones-matmul for
                                  the cross-partition reduction)
    OT_h = V_h^T P_h / colsum    ([n, s])                       (fp16 matmuls)
    Z_h  = O_h Wp_h  (partial output [s, m])                    (fp16 matmuls)
The host passes x pre-transposed ([E, B*S]) so no on-device transposes are
needed anywhere; the partials are summed on the host and bp is added.
Projections read x / Wq / Wk / Wv as float32r (full fp32 bytes, PE rounds
internally, 1 cycle/row at N>=256); Q/K/V/P are stored fp16 in SBUF so both
K_b and V_b stay resident per batch. All PSUM accumulation is fp32.
"""

import numpy as np

import concourse.bacc as bacc
import concourse.mybir as mybir
import concourse.tile as tile
from concourse.bass import ds, ts
from concourse.bass_utils import run_bass_kernel_spmd

H = 8
E = 768
B = 4
S = 2048
TOK = B * S          # 8192 tokens
P = 128              # partitions
EC = E // P          # 6 chunks of the embedding dim
SC = 512             # s-chunk (query block, one PSUM bank wide)
NSC = S // SC        # 4 s-chunks per batch
NT = S // P          # 16 key tiles per batch
VN = 384             # V / Z free-dim chunk (768 = 2 x 384, >=256 keeps f32r fast)

F32 = mybir.dt.float32
F32R = mybir.dt.float32r
F16 = mybir.dt.float16

_NC_CACHE = None


def _build_nc():
    nc = bacc.Bacc("TRN2", target_bir_lowering=False, debug=False, num_devices=H)

    xT = nc.dram_tensor("xT", [E, TOK], F32R, kind="ExternalInput")
    a = nc.dram_tensor("a", [E, E], F32R, kind="ExternalInput")
    wv = nc.dram_tensor("wv", [E, E], F32R, kind="ExternalInput")
    out = nc.dram_tensor("out", [TOK, E], F32, kind="ExternalOutput")

    xT3 = xT[:].rearrange("(eo ei) t -> ei eo t", ei=P)
    a3 = a[:].rearrange("(eo ei) f -> ei eo f", ei=P)
    wv3 = wv[:].rearrange("(eo ei) d -> ei eo d", ei=P)

    inv_sqrt_e = float(1.0 / np.sqrt(E))

    with tile.TileContext(nc) as tc:
        with (
            tc.tile_pool(name="wpool", bufs=1) as wpool,
            tc.tile_pool(name="kvpool", bufs=1) as kvpool,
            tc.tile_pool(name="work", bufs=2) as work,
            tc.tile_pool(name="pexps", bufs=34) as pexps,
            tc.tile_pool(name="zs", bufs=3) as zs,
            tc.tile_pool(name="ps_proj", bufs=3, space="PSUM") as ps_proj,
            tc.tile_pool(name="ps_sc", bufs=2, space="PSUM") as ps_sc,
            tc.tile_pool(name="ps_cs", bufs=1, space="PSUM") as ps_cs,
            tc.tile_pool(name="ps_ot", bufs=2, space="PSUM") as ps_ot,
        ):
            a_sb = wpool.tile([P, EC, E], F32R, name="a_sb")
            wv_sb = wpool.tile([P, EC, E], F32R, name="wv_sb")
            # DMA issue order: first x chunk + wv first half gate the first
            # V-proj group; a/wp are deferred to phase 2.
            xtb = {}
            xtb[(0, 0)] = work.tile([P, EC, SC], F32R, tag="xtb", bufs=5,
                                    name="xtb_0_0")
            nc.sync.dma_start(xtb[(0, 0)][:], xT3[:, :, ds(0, SC)])
            for nch in range(E // VN):
                nc.sync.dma_start(
                    wv_sb[:, :, ds(nch * VN, VN)], wv3[:, :, ds(nch * VN, VN)]
                )
            ones_f32 = wpool.tile([P, P], F32, name="ones_f32")
            nc.vector.memset(ones_f32[:], 1.0)
            ones = wpool.tile([P, P], F32R, name="ones")
            nc.vector.tensor_copy(out=ones[:], in_=ones_f32[:])

            # Warm the PE (HAM clock ramp) with throwaway matmuls while the
            # first weight/x DMAs are in flight, so real matmuls start at the
            # full 2.4 GHz rate.
            for w in range(26):
                pw = ps_cs.tile([P, P], F32, tag="ps_cs", name="pw")
                nc.tensor.matmul(pw[:], ones[:], ones[:], start=True, stop=True)

            for b in range(B):
                tok0 = b * S
                v = kvpool.tile([P, NT, E], F16, tag="v", name=f"v_{b}")

                # ---- phase 1: V_b (x chunks stay resident for scores) ----
                for tci in range(NSC):
                    if (b, tci) not in xtb:
                        xtb[(b, tci)] = work.tile(
                            [P, EC, SC], F32R, tag="xtb", bufs=5,
                            name=f"xtb_{b}_{tci}"
                        )
                        nc.sync.dma_start(
                            xtb[(b, tci)][:], xT3[:, :, ds(tok0 + tci * SC, SC)]
                        )
                    xts = xtb[(b, tci)]
                    # nch outer: consumes wv's first half before the second
                    # arrives at startup
                    for nch in range(E // VN):
                        for tt in range(SC // P):
                            t_tile = tci * (SC // P) + tt
                            pv = ps_proj.tile([P, VN], F32, tag="ps_proj", name="pv")
                            for e in range(EC):
                                nc.tensor.matmul(
                                    pv[:],
                                    xts[:, e, ts(tt, P)],
                                    wv_sb[:, e, ds(nch * VN, VN)],
                                    start=(e == 0),
                                    stop=(e == EC - 1),
                                )
                            nc.vector.tensor_copy(
                                out=v[:, t_tile, ds(nch * VN, VN)], in_=pv[:]
                            )

                # ---- phase 2: attention per s-chunk; PZ of chunk i is
                # emitted after scores of i+1 so the P-normalize (DVE) runs
                # under the next score block instead of stalling the PE ----
                pz_emits = []
                for sci in range(NSC):
                    s0 = tok0 + sci * SC
                    if b == 0 and sci == 0:
                        # deferred weight load: needed from here on
                        nc.sync.dma_start(a_sb[:], a3)
                    # G^T = A^T X^T: the query-side operand; x slice is the
                    # s-chunk of the resident batch chunks (s range == t range)
                    gt = work.tile([P, EC, SC], F32R, tag="gt", name=f"gt_{b}_{sci}")
                    for f in range(EC):
                        pq = ps_proj.tile([P, SC], F32, tag="ps_proj", name="pq")
                        for e in range(EC):
                            nc.tensor.matmul(
                                pq[:],
                                a_sb[:, e, ts(f, P)],
                                xtb[(b, sci)][:, e, :],
                                start=(e == 0),
                                stop=(e == EC - 1),
                            )
                        nc.vector.tensor_copy(out=gt[:, f, :], in_=pq[:])

                    # scores + exp; partial column sums accumulate on DVE in
                    # f32r; one f32r ones-matmul then reduces across
                    # partitions (replaces 16 PE colsum matmuls per s-chunk)
                    csum = work.tile([P, SC], F32R, tag="csum", name="csum", bufs=1)
                    pexp_tiles = []
                    for t in range(NT):
                        pst = ps_sc.tile([P, SC], F32, tag="ps_sc", name="pst")
                        for f in range(EC):
                            nc.tensor.matmul(
                                pst[:],
                                xtb[(b, t // 4)][:, f, ts(t % 4, P)],
                                gt[:, f, :],
                                start=(f == 0),
                                stop=(f == EC - 1),
                            )
                        pe_t = pexps.tile([P, SC], F16, tag="pexp", name=f"pexp_{t}")
                        nc.scalar.activation(
                            pe_t[:],
                            pst[:],
                            mybir.ActivationFunctionType.Exp,
                            scale=inv_sqrt_e,
                        )
                        pexp_tiles.append(pe_t)
                        if t == 0:
                            nc.vector.tensor_copy(out=csum[:], in_=pe_t[:])
                        else:
                            nc.vector.tensor_add(
                                out=csum[:], in0=csum[:], in1=pe_t[:]
                            )
                    pcs = ps_cs.tile([P, SC], F32, tag="ps_cs", name="pcs")
                    nc.tensor.matmul(
                        pcs[:], ones[:], csum[:], start=True, stop=True
                    )
                    rec = work.tile([P, SC], F32, tag="rec", name="rec", bufs=1)
                    nc.vector.reciprocal(rec[:], pcs[:])
                    for t in range(NT):
                        nc.vector.tensor_mul(
                            out=pexp_tiles[t][:], in0=pexp_tiles[t][:], in1=rec[:]
                        )

                    # Z = P-hat^T U directly (U = X Wv Wp resident as `v`;
                    # O is never materialized)
                    def emit_pz(s0=s0, pexp_tiles=pexp_tiles, v=v):
                        for st in range(SC // P):
                            for mch in range(E // VN):
                                pz = ps_ot.tile([P, VN], F32, tag="ps_ot", name="pz")
                                for t in range(NT):
                                    nc.tensor.matmul(
                                        pz[:],
                                        pexp_tiles[t][:, ts(st, P)],
                                        v[:, t, ds(mch * VN, VN)],
                                        start=(t == 0),
                                        stop=(t == NT - 1),
                                    )
                                z = zs.tile([P, VN], F32, tag="z", name="z")
                                nc.vector.tensor_copy(out=z[:], in_=pz[:])
                                nc.sync.dma_start(
                                    out[ds(s0 + st * P, P), ds(mch * VN, VN)], z[:]
                                )

                    pz_emits.append(emit_pz)
                    if sci > 0:
                        pz_emits[sci - 1]()
                for_last = pz_emits[NSC - 1]
                for_last()

    nc.compile()
    return nc


def get_nc():
    global _NC_CACHE
    if _NC_CACHE is None:
        _NC_CACHE = _build_nc()
    return _NC_CACHE


def make_in_maps(x, Wq, Wk, Wv, Wp):
    x = np.asarray(x, dtype=np.float32)
    Wq = np.asarray(Wq, dtype=np.float32)
    Wk = np.asarray(Wk, dtype=np.float32)
    Wv = np.asarray(Wv, dtype=np.float32)
    Wp = np.asarray(Wp, dtype=np.float32)
    xT = np.ascontiguousarray(x.reshape(TOK, E).T)
    in_maps = []
    for h in range(H):
        # A_h[e, f] = sum_d Wq_h[e, d] Wk_h[f, d]: collapses the Q and K
        # projections into one on-device G = X @ A projection.
        a_h = np.ascontiguousarray(Wq[h] @ Wk[h].T)
        # C_h = Wv_h @ Wp_h folds the value and output projections: the
        # device computes U = X @ C_h once and Z = P_hat^T U directly.
        c_h = np.ascontiguousarray(Wv[h] @ Wp[h * E : (h + 1) * E])
        in_maps.append(
            {
                "xT": xT,
                "a": a_h,
                "wv": c_h,
            }
        )
    return in_maps


def kernel(x, Wq, Wk, Wv, Wp, bp):
    nc = get_nc()
    in_maps = make_in_maps(x, Wq, Wk, Wv, Wp)
    res = run_bass_kernel_spmd(nc, in_maps, core_ids=list(range(H)))
    acc = res.results[0]["out"].copy()
    for h in range(1, H):
        acc += res.results[h]["out"]
    acc += np.asarray(bp, dtype=np.float32)
    return acc.reshape(B, S, E)


# revision 36
# speedup vs baseline: 1.0686x; 1.0686x over previous
"""Trainium2 Bass kernel for nn_MultiHeadAttention_6055903887702.

Sharding: one attention head per NeuronCore (H == n_cores == 8). Each core
computes, for its head h:
    A_h  = Wq_h Wk_h^T  (host-precomputed, so Q/K projections collapse)
    GT_h = A_h^T X^T, V_h = X Wv_h                              (f32r matmuls)
    ST_h = X G^T      (scores, transposed layout [t, s])        (f32r matmuls)
    P_h  = exp(ST_h / sqrt(E))   (unnormalized, no max-sub — logits ~N(0,1))
    colsum[s] = sum_t P_h[t, s]  (DVE accumulate + one f32r ones-matmul for
                                  the cross-partition reduction)
    OT_h = V_h^T P_h / colsum    ([n, s])                       (fp16 matmuls)
    Z_h  = O_h Wp_h  (partial output [s, m])                    (fp16 matmuls)
The host passes x pre-transposed ([E, B*S]) so no on-device transposes are
needed anywhere; the partials are summed on the host and bp is added.
Projections read x / Wq / Wk / Wv as float32r (full fp32 bytes, PE rounds
internally, 1 cycle/row at N>=256); Q/K/V/P are stored fp16 in SBUF so both
K_b and V_b stay resident per batch. All PSUM accumulation is fp32.
"""

import numpy as np

import concourse.bacc as bacc
import concourse.mybir as mybir
import concourse.tile as tile
from concourse.bass import ds, ts
from concourse.bass_utils import run_bass_kernel_spmd

H = 8
E = 768
B = 4
S = 2048
TOK = B * S          # 8192 tokens
P = 128              # partitions
EC = E // P          # 6 chunks of the embedding dim
SC = 512             # s-chunk (query block, one PSUM bank wide)
NSC = S // SC        # 4 s-chunks per batch
NT = S // P          # 16 key tiles per batch
VN = 384             # V / Z free-dim chunk (768 = 2 x 384, >=256 keeps f32r fast)

F32 = mybir.dt.float32
F32R = mybir.dt.float32r
F16 = mybir.dt.float16

_NC_CACHE = None


def _build_nc():
    nc = bacc.Bacc("TRN2", target_bir_lowering=False, debug=False, num_devices=H)

    xT = nc.dram_tensor("xT", [E, TOK], F32R, kind="ExternalInput")
    a = nc.dram_tensor("a", [E, E], F32R, kind="ExternalInput")
    wv = nc.dram_tensor("wv", [E, E], F32R, kind="ExternalInput")
    out = nc.dram_tensor("out", [TOK, E], F32, kind="ExternalOutput")

    xT3 = xT[:].rearrange("(eo ei) t -> ei eo t", ei=P)
    a3 = a[:].rearrange("(eo ei) f -> ei eo f", ei=P)
    wv3 = wv[:].rearrange("(eo ei) d -> ei eo d", ei=P)

    inv_sqrt_e = float(1.0 / np.sqrt(E))

    with tile.TileContext(nc) as tc:
        with (
            tc.tile_pool(name="wpool", bufs=1) as wpool,
            tc.tile_pool(name="kvpool", bufs=1) as kvpool,
            tc.tile_pool(name="work", bufs=2) as work,
            tc.tile_pool(name="pexps", bufs=18) as pexps,
            tc.tile_pool(name="zs", bufs=3) as zs,
            tc.tile_pool(name="ps_proj", bufs=3, space="PSUM") as ps_proj,
            tc.tile_pool(name="ps_sc", bufs=2, space="PSUM") as ps_sc,
            tc.tile_pool(name="ps_cs", bufs=1, space="PSUM") as ps_cs,
            tc.tile_pool(name="ps_ot", bufs=2, space="PSUM") as ps_ot,
        ):
            a_sb = wpool.tile([P, EC, E], F32R, name="a_sb")
            wv_sb = wpool.tile([P, EC, E], F32R, name="wv_sb")
            # DMA issue order: first x chunk + wv first half gate the first
            # V-proj group; a/wp are deferred to phase 2.
            xtb = {}
            xtb[(0, 0)] = work.tile([P, EC, SC], F32R, tag="xtb", bufs=5,
                                    name="xtb_0_0")
            nc.sync.dma_start(xtb[(0, 0)][:], xT3[:, :, ds(0, SC)])
            for nch in range(E // VN):
                nc.sync.dma_start(
                    wv_sb[:, :, ds(nch * VN, VN)], wv3[:, :, ds(nch * VN, VN)]
                )
            ones_f32 = wpool.tile([P, P], F32, name="ones_f32")
            nc.vector.memset(ones_f32[:], 1.0)
            ones = wpool.tile([P, P], F32R, name="ones")
            nc.vector.tensor_copy(out=ones[:], in_=ones_f32[:])

            # Warm the PE (HAM clock ramp) with throwaway matmuls while the
            # first weight/x DMAs are in flight, so real matmuls start at the
            # full 2.4 GHz rate.
            for w in range(26):
                pw = ps_cs.tile([P, P], F32, tag="ps_cs", name="pw")
                nc.tensor.matmul(pw[:], ones[:], ones[:], start=True, stop=True)

            for b in range(B):
                tok0 = b * S
                v = kvpool.tile([P, NT, E], F16, tag="v", name=f"v_{b}")

                # ---- phase 1: V_b (x chunks stay resident for scores) ----
                for tci in range(NSC):
                    if (b, tci) not in xtb:
                        xtb[(b, tci)] = work.tile(
                            [P, EC, SC], F32R, tag="xtb", bufs=5,
                            name=f"xtb_{b}_{tci}"
                        )
                        nc.sync.dma_start(
                            xtb[(b, tci)][:], xT3[:, :, ds(tok0 + tci * SC, SC)]
                        )
                    xts = xtb[(b, tci)]
                    # nch outer: consumes wv's first half before the second
                    # arrives at startup
                    for nch in range(E // VN):
                        for tt in range(SC // P):
                            t_tile = tci * (SC // P) + tt
                            pv = ps_proj.tile([P, VN], F32, tag="ps_proj", name="pv")
                            for e in range(EC):
                                nc.tensor.matmul(
                                    pv[:],
                                    xts[:, e, ts(tt, P)],
                                    wv_sb[:, e, ds(nch * VN, VN)],
                                    start=(e == 0),
                                    stop=(e == EC - 1),
                                )
                            nc.vector.tensor_copy(
                                out=v[:, t_tile, ds(nch * VN, VN)], in_=pv[:]
                            )

                # ---- phase 2: attention per s-chunk ----
                for sci in range(NSC):
                    s0 = tok0 + sci * SC
                    if b == 0 and sci == 0:
                        # deferred weight load: needed from here on
                        nc.sync.dma_start(a_sb[:], a3)
                    # G^T = A^T X^T: the query-side operand; x slice is the
                    # s-chunk of the resident batch chunks (s range == t range)
                    gt = work.tile([P, EC, SC], F32R, tag="gt", name=f"gt_{b}_{sci}")
                    for f in range(EC):
                        pq = ps_proj.tile([P, SC], F32, tag="ps_proj", name="pq")
                        for e in range(EC):
                            nc.tensor.matmul(
                                pq[:],
                                a_sb[:, e, ts(f, P)],
                                xtb[(b, sci)][:, e, :],
                                start=(e == 0),
                                stop=(e == EC - 1),
                            )
                        nc.vector.tensor_copy(out=gt[:, f, :], in_=pq[:])

                    # scores + exp; partial column sums accumulate on DVE in
                    # f32r; one f32r ones-matmul then reduces across
                    # partitions (replaces 16 PE colsum matmuls per s-chunk)
                    csum = work.tile([P, SC], F32R, tag="csum", name="csum", bufs=1)
                    pexp_tiles = []
                    for t in range(NT):
                        pst = ps_sc.tile([P, SC], F32, tag="ps_sc", name="pst")
                        for f in range(EC):
                            nc.tensor.matmul(
                                pst[:],
                                xtb[(b, t // 4)][:, f, ts(t % 4, P)],
                                gt[:, f, :],
                                start=(f == 0),
                                stop=(f == EC - 1),
                            )
                        pe_t = pexps.tile([P, SC], F16, tag="pexp", name=f"pexp_{t}")
                        nc.scalar.activation(
                            pe_t[:],
                            pst[:],
                            mybir.ActivationFunctionType.Exp,
                            scale=inv_sqrt_e,
                        )
                        pexp_tiles.append(pe_t)
                        if t == 0:
                            nc.vector.tensor_copy(out=csum[:], in_=pe_t[:])
                        else:
                            nc.vector.tensor_add(
                                out=csum[:], in0=csum[:], in1=pe_t[:]
                            )
                    pcs = ps_cs.tile([P, SC], F32, tag="ps_cs", name="pcs")
                    nc.tensor.matmul(
                        pcs[:], ones[:], csum[:], start=True, stop=True
                    )
                    rec = work.tile([P, SC], F32, tag="rec", name="rec", bufs=1)
                    nc.vector.reciprocal(rec[:], pcs[:])
                    for t in range(NT):
                        nc.vector.tensor_mul(
                            out=pexp_tiles[t][:], in0=pexp_tiles[t][:], in1=rec[:]
                        )

                    # Z = P-hat^T U directly (U = X Wv Wp resident as `v`;
                    # O is never materialized)
                    for st in range(SC // P):
                        for mch in range(E // VN):
                            pz = ps_ot.tile([P, VN], F32, tag="ps_ot", name="pz")
                            for t in range(NT):
                                nc.tensor.matmul(
                                    pz[:],
                                    pexp_tiles[t][:, ts(st, P)],
                                    v[:, t, ds(mch * VN, VN)],
                                    start=(t == 0),
                                    stop=(t == NT - 1),
                                )
                            z = zs.tile([P, VN], F32, tag="z", name="z")
                            nc.vector.tensor_copy(out=z[:], in_=pz[:])
                            nc.sync.dma_start(
                                out[ds(s0 + st * P, P), ds(mch * VN, VN)], z[:]
                            )

    nc.compile()
    return nc


def get_nc():
    global _NC_CACHE
    if _NC_CACHE is None:
        _NC_CACHE = _build_nc()
    return _NC_CACHE


def make_in_maps(x, Wq, Wk, Wv, Wp):
    x = np.asarray(x, dtype=np.float32)
    Wq = np.asarray(Wq, dtype=np.float32)
    Wk = np.asarray(Wk, dtype=np.float32)
    Wv = np.asarray(Wv, dtype=np.float32)
    Wp = np.asarray(Wp, dtype=np.float32)
    xT = np.ascontiguousarray(x.reshape(TOK, E).T)
    in_maps = []
    for h in range(H):
        # A_h[e, f] = sum_d Wq_h[e, d] Wk_h[f, d]: collapses the Q and K
        # projections into one on-device G = X @ A projection.
        a_h = np.ascontiguousarray(Wq[h] @ Wk[h].T)
        # C_h = Wv_h @ Wp_h folds the value and output projections: the
        # device computes U = X @ C_h once and Z = P_hat^T U directly.
        c_h = np.ascontiguousarray(Wv[h] @ Wp[h * E : (h + 1) * E])
        in_maps.append(
            {
                "xT": xT,
                "a": a_h,
                "wv": c_h,
            }
        )
    return in_maps


def kernel(x, Wq, Wk, Wv, Wp, bp):
    nc = get_nc()
    in_maps = make_in_maps(x, Wq, Wk, Wv, Wp)
    res = run_bass_kernel_spmd(nc, in_maps, core_ids=list(range(H)))
    acc = res.results[0]["out"].copy()
    for h in range(1, H):
        acc += res.results[h]["out"]
    acc += np.asarray(bp, dtype=np.float32)
    return acc.reshape(B, S, E)


# revision 37
# speedup vs baseline: 1.1221x; 1.0501x over previous
"""Trainium2 Bass kernel for nn_MultiHeadAttention_6055903887702.

Sharding: one attention head per NeuronCore (H == n_cores == 8). Each core
computes, for its head h:
    A_h  = Wq_h Wk_h^T  (host-precomputed, so Q/K projections collapse)
    GT_h = A_h^T X^T, V_h = X Wv_h                              (f32r matmuls)
    ST_h = X G^T      (scores, transposed layout [t, s])        (f32r matmuls)
    P_h  = exp(ST_h / sqrt(E))   (unnormalized, no max-sub — logits ~N(0,1))
    colsum[s] = sum_t P_h[t, s]  (DVE accumulate + one f32r ones-matmul for
                                  the cross-partition reduction)
    OT_h = V_h^T P_h / colsum    ([n, s])                       (fp16 matmuls)
    Z_h  = O_h Wp_h  (partial output [s, m])                    (fp16 matmuls)
The host passes x pre-transposed ([E, B*S]) so no on-device transposes are
needed anywhere; the partials are summed on the host and bp is added.
Projections read x / Wq / Wk / Wv as float32r (full fp32 bytes, PE rounds
internally, 1 cycle/row at N>=256); Q/K/V/P are stored fp16 in SBUF so both
K_b and V_b stay resident per batch. All PSUM accumulation is fp32.
"""

import numpy as np

import concourse.bacc as bacc
import concourse.mybir as mybir
import concourse.tile as tile
from concourse.bass import ds, ts
from concourse.bass_utils import run_bass_kernel_spmd
from concourse.masks import make_identity

H = 8
E = 768
B = 4
S = 2048
TOK = B * S          # 8192 tokens
P = 128              # partitions
EC = E // P          # 6 chunks of the embedding dim
SC = 512             # s-chunk (query block, one PSUM bank wide)
NSC = S // SC        # 4 s-chunks per batch
NT = S // P          # 16 key tiles per batch
VN = 384             # V / Z free-dim chunk (768 = 2 x 384, >=256 keeps f32r fast)

F32 = mybir.dt.float32
F32R = mybir.dt.float32r
F16 = mybir.dt.float16

_NC_CACHE = None


def _build_nc():
    nc = bacc.Bacc("TRN2", target_bir_lowering=False, debug=False, num_devices=H)

    xT = nc.dram_tensor("xT", [E, TOK], F32R, kind="ExternalInput")
    a = nc.dram_tensor("a", [E, E], F32R, kind="ExternalInput")
    wv = nc.dram_tensor("wv", [E, E], F32R, kind="ExternalInput")
    out = nc.dram_tensor("out", [TOK, E], F32, kind="ExternalOutput")

    xT3 = xT[:].rearrange("(eo ei) t -> ei eo t", ei=P)
    a3 = a[:].rearrange("(eo ei) f -> ei eo f", ei=P)
    wv3 = wv[:].rearrange("(eo ei) d -> ei eo d", ei=P)

    inv_sqrt_e = float(1.0 / np.sqrt(E))

    with tile.TileContext(nc) as tc:
        with (
            tc.tile_pool(name="wpool", bufs=1) as wpool,
            tc.tile_pool(name="kvpool", bufs=1) as kvpool,
            tc.tile_pool(name="work", bufs=2) as work,
            tc.tile_pool(name="pexps", bufs=18) as pexps,
            tc.tile_pool(name="zs", bufs=3) as zs,
            tc.tile_pool(name="ps_proj", bufs=3, space="PSUM") as ps_proj,
            tc.tile_pool(name="ps_sc", bufs=2, space="PSUM") as ps_sc,
            tc.tile_pool(name="ps_cs", bufs=1, space="PSUM") as ps_cs,
            tc.tile_pool(name="ps_ot", bufs=2, space="PSUM") as ps_ot,
        ):
            a_sb = wpool.tile([P, EC, E], F32R, name="a_sb")
            wv_sb = wpool.tile([P, EC, E], F32R, name="wv_sb")
            # DMA issue order: first x chunk + wv first half gate the first
            # V-proj group; a/wp are deferred to phase 2.
            xtb = {}
            xtb[(0, 0)] = work.tile([P, EC, SC], F32R, tag="xtb", bufs=5,
                                    name="xtb_0_0")
            nc.sync.dma_start(xtb[(0, 0)][:], xT3[:, :, ds(0, SC)])
            for nch in range(E // VN):
                nc.sync.dma_start(
                    wv_sb[:, :, ds(nch * VN, VN)], wv3[:, :, ds(nch * VN, VN)]
                )
            ident = wpool.tile([P, P], F32, name="ident")
            make_identity(nc, ident[:])
            ones_f32 = wpool.tile([P, P], F32, name="ones_f32")
            nc.vector.memset(ones_f32[:], 1.0)
            ones = wpool.tile([P, P], F32R, name="ones")
            nc.vector.tensor_copy(out=ones[:], in_=ones_f32[:])

            # Warm the PE (HAM clock ramp) with throwaway matmuls while the
            # first weight/x DMAs are in flight, so real matmuls start at the
            # full 2.4 GHz rate.
            for w in range(26):
                pw = ps_cs.tile([P, P], F32, tag="ps_cs", name="pw")
                nc.tensor.matmul(pw[:], ones[:], ones[:], start=True, stop=True)

            for b in range(B):
                tok0 = b * S
                v = kvpool.tile([P, NT, E], F16, tag="v", name=f"v_{b}")

                # ---- phase 1: V_b (x chunks stay resident for scores) ----
                for tci in range(NSC):
                    if (b, tci) not in xtb:
                        xtb[(b, tci)] = work.tile(
                            [P, EC, SC], F32R, tag="xtb", bufs=5,
                            name=f"xtb_{b}_{tci}"
                        )
                        nc.sync.dma_start(
                            xtb[(b, tci)][:], xT3[:, :, ds(tok0 + tci * SC, SC)]
                        )
                    xts = xtb[(b, tci)]
                    # nch outer: consumes wv's first half before the second
                    # arrives at startup
                    for nch in range(E // VN):
                        for tt in range(SC // P):
                            t_tile = tci * (SC // P) + tt
                            pv = ps_proj.tile([P, VN], F32, tag="ps_proj", name="pv")
                            for e in range(EC):
                                nc.tensor.matmul(
                                    pv[:],
                                    xts[:, e, ts(tt, P)],
                                    wv_sb[:, e, ds(nch * VN, VN)],
                                    start=(e == 0),
                                    stop=(e == EC - 1),
                                )
                            nc.vector.tensor_copy(
                                out=v[:, t_tile, ds(nch * VN, VN)], in_=pv[:]
                            )

                # ---- phase 2: attention per s-chunk ----
                for sci in range(NSC):
                    s0 = tok0 + sci * SC
                    if b == 0 and sci == 0:
                        # deferred weight load: needed from here on
                        nc.sync.dma_start(a_sb[:], a3)
                    # G^T = A^T X^T: the query-side operand; x slice is the
                    # s-chunk of the resident batch chunks (s range == t range)
                    gt = work.tile([P, EC, SC], F32R, tag="gt", name=f"gt_{b}_{sci}")
                    for f in range(EC):
                        pq = ps_proj.tile([P, SC], F32, tag="ps_proj", name="pq")
                        for e in range(EC):
                            nc.tensor.matmul(
                                pq[:],
                                a_sb[:, e, ts(f, P)],
                                xtb[(b, sci)][:, e, :],
                                start=(e == 0),
                                stop=(e == EC - 1),
                            )
                        nc.vector.tensor_copy(out=gt[:, f, :], in_=pq[:])

                    # scores + exp; partial column sums accumulate on DVE in
                    # f32r; one f32r ones-matmul then reduces across
                    # partitions (replaces 16 PE colsum matmuls per s-chunk)
                    csum = work.tile([P, SC], F32R, tag="csum", name="csum", bufs=1)
                    pexp_tiles = []
                    for t in range(NT):
                        pst = ps_sc.tile([P, SC], F32, tag="ps_sc", name="pst")
                        for f in range(EC):
                            nc.tensor.matmul(
                                pst[:],
                                xtb[(b, t // 4)][:, f, ts(t % 4, P)],
                                gt[:, f, :],
                                start=(f == 0),
                                stop=(f == EC - 1),
                            )
                        pe_t = pexps.tile([P, SC], F16, tag="pexp", name=f"pexp_{t}")
                        nc.scalar.activation(
                            pe_t[:],
                            pst[:],
                            mybir.ActivationFunctionType.Exp,
                            scale=inv_sqrt_e,
                        )
                        pexp_tiles.append(pe_t)
                        if t == 0:
                            nc.vector.tensor_copy(out=csum[:], in_=pe_t[:])
                        else:
                            nc.vector.tensor_add(
                                out=csum[:], in0=csum[:], in1=pe_t[:]
                            )
                    pcs = ps_cs.tile([P, SC], F32, tag="ps_cs", name="pcs")
                    nc.tensor.matmul(
                        pcs[:], ones[:], csum[:], start=True, stop=True
                    )
                    rec = work.tile([P, SC], F32, tag="rec", name="rec", bufs=2)
                    nc.vector.reciprocal(rec[:], pcs[:])

                    # Z = P-hat^T U directly (U = X Wv Wp resident as `v`;
                    # O is never materialized)
                    rec_col = work.tile([P, NSC], F32, tag="rec_col",
                                        name="rec_col", bufs=2)
                    first = True
                    for st in range(SC // P):
                        for mch in range(E // VN):
                            pz = ps_ot.tile([P, VN], F32, tag="ps_ot", name="pz")
                            for t in range(NT):
                                nc.tensor.matmul(
                                    pz[:],
                                    pexp_tiles[t][:, ts(st, P)],
                                    v[:, t, ds(mch * VN, VN)],
                                    start=(t == 0),
                                    stop=(t == NT - 1),
                                )
                            if first:
                                # bring 1/colsum to per-partition layout via PE
                                # transposes (off the critical z-mult path)
                                first = False
                                for st2 in range(SC // P):
                                    tp = ps_cs.tile([P, P], F32, tag="ps_cs",
                                                    name="tp")
                                    nc.tensor.transpose(
                                        tp[:], rec[:, ts(st2, P)], ident[:]
                                    )
                                    nc.vector.tensor_copy(
                                        out=rec_col[:, st2 : st2 + 1],
                                        in_=tp[:, 0:1],
                                    )
                            z = zs.tile([P, VN], F32, tag="z", name="z")
                            nc.vector.tensor_scalar_mul(
                                z[:], pz[:], rec_col[:, st : st + 1]
                            )
                            nc.sync.dma_start(
                                out[ds(s0 + st * P, P), ds(mch * VN, VN)], z[:]
                            )

    nc.compile()
    return nc


def get_nc():
    global _NC_CACHE
    if _NC_CACHE is None:
        _NC_CACHE = _build_nc()
    return _NC_CACHE


def make_in_maps(x, Wq, Wk, Wv, Wp):
    x = np.asarray(x, dtype=np.float32)
    Wq = np.asarray(Wq, dtype=np.float32)
    Wk = np.asarray(Wk, dtype=np.float32)
    Wv = np.asarray(Wv, dtype=np.float32)
    Wp = np.asarray(Wp, dtype=np.float32)
    xT = np.ascontiguousarray(x.reshape(TOK, E).T)
    in_maps = []
    for h in range(H):
        # A_h[e, f] = sum_d Wq_h[e, d] Wk_h[f, d]: collapses the Q and K
        # projections into one on-device G = X @ A projection.
        a_h = np.ascontiguousarray(Wq[h] @ Wk[h].T)
        # C_h = Wv_h @ Wp_h folds the value and output projections: the
        # device computes U = X @ C_h once and Z = P_hat^T U directly.
        c_h = np.ascontiguousarray(Wv[h] @ Wp[h * E : (h + 1) * E])
        in_maps.append(
            {
                "xT": xT,
                "a": a_h,
                "wv": c_h,
            }
        )
    return in_maps


def kernel(x, Wq, Wk, Wv, Wp, bp):
    nc = get_nc()
    in_maps = make_in_maps(x, Wq, Wk, Wv, Wp)
    res = run_bass_kernel_spmd(nc, in_maps, core_ids=list(range(H)))
    acc = res.results[0]["out"].copy()
    for h in range(1, H):
        acc += res.results[h]["out"]
    acc += np.asarray(bp, dtype=np.float32)
    return acc.reshape(B, S, E)
